# revision 1
# baseline (speedup 1.0000x reference)
"""BEV voxel-pooling (segment_reduce) kernel for 8 Trainium2 NeuronCores.

Strategy
--------
Host (numpy, cheap — driven only by the small geometry inputs):
  * compute each point's BEV rank (bin id) exactly as the reference does
  * per sample, stable-sort points by rank; split the sorted stream into 4
    shards of ~equal point count snapped to rank boundaries (8 shards total
    across B=2 samples -> 8 cores, disjoint rank ranges)
  * per core, pack points into 128-point chunks grouped by "segment blocks"
    (128 distinct ranks per block); upload the permuted features as an
    fp16 hi/lo pair (x ~= hi + lo, error ~2^-24 — f32-class accuracy)

Device (per core, one SPMD Bass/Tile program):
  * stream feature chunks in; build a per-chunk one-hot (point -> local
    segment) on the DVE via iota/is_equal; two fp16 matmuls per chunk
    (hi and lo) accumulate segment sums into a PSUM-resident accumulator
    [128 segs x nblocks*64ch] at a per-group dynamic column offset
  * PSUM is pre-zeroed with K=1 start=True dummy matmuls (keeps all PSUM
    dependencies on the PE; walrus rejects multi-wait compute instructions)
  * copy PSUM -> SBUF once at the end (ACT) and dma_scatter_add the segment
    rows into the per-core output slice [span, 64] (output buffers are
    pre-zeroed by the runtime; scatter destinations are unique)

Host gather: place each core's [span, 64] rows into the (B, 40000, 64) grid,
reshape to the reference layout (B, C, X, Y).
"""
import sys
sys.path.insert(0, '/opt/trn_rl_repo')

import numpy as np

# ---------------- problem constants (hardcoded per spec) ----------------
B, N, C = 2, 6, 64
H_IMG, W_IMG = 256, 704
DS = 16
DSH, DSW = H_IMG // DS, W_IMG // DS          # 16, 44
D0, D1 = 4, 45                                # depth bins -> D = 41
X, Y, Z = 200, 200, 1
NBINS = X * Y * Z                             # 40000
NP_SAMPLE = N * (D1 - D0) * DSH * DSW         # 173184
NCORES = 8
SHARDS_PER_SAMPLE = 4

V = 2            # chunks per PSUM accumulation group
ABS_EVERY = 8    # absorber cadence, in groups

_compiled = {}


# ---------------- host geometry (matches reference numerics) ----------------
def _compute_ranks(frustum, post_trans, post_rots, intrinsics, extrinsics,
                   bev_res, bev_start_pos):
    frustum = np.asarray(frustum, np.float32)
    post_trans = np.asarray(post_trans, np.float32)
    post_rots = np.asarray(post_rots, np.float32)
    intrinsics = np.asarray(intrinsics, np.float32)
    extrinsics = np.asarray(extrinsics, np.float32)
    bev_res = np.asarray(bev_res, np.float32)
    bev_start_pos = np.asarray(bev_start_pos, np.float32)

    ext_inv = np.linalg.inv(extrinsics.astype(np.float64)).astype(np.float32)
    rot = ext_inv[..., :3, :3]
    trans = ext_inv[..., :3, 3]
    pts = frustum[None, None] - post_trans[:, :, None, None, None, :]
    pr_inv = np.linalg.inv(post_rots.astype(np.float64)).astype(np.float32)
    pts = np.einsum('bnij,bndhwj->bndhwi', pr_inv, pts).astype(np.float32)
    pts = np.concatenate([pts[..., :2] * pts[..., 2:3], pts[..., 2:3]], axis=-1)
    comb = (rot @ np.linalg.inv(intrinsics.astype(np.float64)).astype(np.float32)
            ).astype(np.float32)
    pts = np.einsum('bnij,bndhwj->bndhwi', comb, pts).astype(np.float32)
    geom = pts + trans[:, :, None, None, None, :]

    coords = (geom - (bev_start_pos - bev_res / 2.0)) / bev_res
    ci = coords.reshape(B, -1, 3).astype(np.int32)
    mask = ((ci[..., 0] >= 0) & (ci[..., 0] < X) &
            (ci[..., 1] >= 0) & (ci[..., 1] < Y) &
            (ci[..., 2] >= 0) & (ci[..., 2] < Z))
    rank = ci[..., 0] * (Y * Z) + ci[..., 1] * Z + ci[..., 2]
    return rank, mask


# ---------------- host planning ----------------
class CorePlan:
    __slots__ = ("order", "ranks_sorted", "lo", "seg_ranks", "nsegs",
                 "chunk_pts", "chunk_lseg", "group_block", "nchunk", "ngroups",
                 "nblocks", "span", "sample")


def _plan_cores(rank, mask, feats):
    """feats: (B, NP_SAMPLE, C) float32. Returns plans + global dims."""
    plans = []
    for b in range(B):
        r = rank[b]
        m = mask[b]
        valid_idx = np.nonzero(m)[0]
        order = valid_idx[np.argsort(r[valid_idx], kind='stable')]
        rs = r[order]
        P = len(order)
        # shard boundaries at rank changes, ~equal points
        cuts = [0]
        for s in range(1, SHARDS_PER_SAMPLE):
            i = s * P // SHARDS_PER_SAMPLE
            while i < P and rs[i] == rs[i - 1]:
                i += 1
            cuts.append(i)
        cuts.append(P)
        for s in range(SHARDS_PER_SAMPLE):
            pl = CorePlan()
            pl.sample = b
            pl.order = order[cuts[s]:cuts[s + 1]]
            pl.ranks_sorted = rs[cuts[s]:cuts[s + 1]]
            plans.append(pl)

    for pl in plans:
        rs = pl.ranks_sorted
        P = len(rs)
        # segment ids (dense, sorted)
        newseg = np.r_[True, rs[1:] != rs[:-1]]
        seg_of_pt = np.cumsum(newseg) - 1
        pl.nsegs = int(seg_of_pt[-1]) + 1 if P else 0
        pl.seg_ranks = rs[newseg]
        pl.lo = int(pl.seg_ranks[0]) if P else 0
        pl.span = int(pl.seg_ranks[-1]) - pl.lo + 1 if P else 1
        nblocks = (pl.nsegs + 127) // 128
        pl.nblocks = nblocks
        block_of_pt = seg_of_pt // 128
        # chunks per block, padded to multiple of V chunks
        chunk_pts = []     # per chunk: np.array of point indices into pl.order
        chunk_lseg = []    # per chunk: np.array [128] of local seg (255 = pad)
        group_block = []   # per group: block id
        for j in range(nblocks):
            sel = np.nonzero(block_of_pt == j)[0]
            nch = max(1, (len(sel) + 127) // 128)
            nch = ((nch + V - 1) // V) * V
            for k in range(nch):
                part = sel[k * 128:(k + 1) * 128]
                ls = np.full(128, 255, np.int32)
                ls[:len(part)] = seg_of_pt[part] - j * 128
                chunk_pts.append(part)
                chunk_lseg.append(ls)
            for g in range(nch // V):
                group_block.append(j)
        pl.chunk_pts = chunk_pts
        pl.chunk_lseg = chunk_lseg
        pl.group_block = group_block
        pl.nchunk = len(chunk_pts)
        pl.ngroups = len(group_block)

    nchunk = max(pl.nchunk for pl in plans)
    ngroups = nchunk // V
    nblocks = max(pl.nblocks for pl in plans) + 1   # +1 dummy block
    span = max(pl.span for pl in plans)
    span_pad = ((span + 127) // 128) * 128 + 1      # +1 dummy row
    return plans, nchunk, ngroups, nblocks, span_pad


def _build_inputs(pl, feats_b, nchunk, ngroups, nblocks, span_pad):
    """Per-core input arrays for the device program."""
    tok = nblocks * 128
    table = np.zeros((nchunk, 128, 2, C), np.float16)   # [c, p, hi/lo, C]
    lseg = np.full((128, nchunk), 255.0, np.float32)
    moff = np.full((1, ngroups), (nblocks - 1) * 128, np.int32)
    idx = np.full(tok, span_pad - 1, np.int16)      # default: dummy row

    for c, (part, ls) in enumerate(zip(pl.chunk_pts, pl.chunk_lseg)):
        if len(part):
            f = feats_b[pl.order[part]]             # [n, C] f32
            hi = f.astype(np.float16)
            lo = (f - hi.astype(np.float32)).astype(np.float16)
            table[c, :len(part), 0] = hi
            table[c, :len(part), 1] = lo
        lseg[:, c] = ls
    for g, j in enumerate(pl.group_block):
        moff[0, g] = j * 128
    idx[:pl.nsegs] = (pl.seg_ranks - pl.lo).astype(np.int16)
    idx_wrapped = np.tile(idx.reshape(tok // 16, 16).T, (8, 1)).copy()

    iota = np.broadcast_to(np.arange(128, dtype=np.float16), (128, 128))
    # partition-major: row p holds all chunks' (hi|lo) rows contiguously
    table_pm = np.ascontiguousarray(table.transpose(1, 0, 2, 3).reshape(128, -1))
    return {
        "table": table_pm,
        "localseg": lseg,
        "iota": np.ascontiguousarray(iota),
        "meta_off": moff,
        "scat_idx": idx_wrapped,
    }


# ---------------- device program ----------------
def _build_kernel(nchunk, ngroups, nblocks, span_pad):
    import concourse.bass as bass
    import concourse.bacc as bacc
    import concourse.mybir as mybir
    import concourse.tile as tile
    from concourse.tile_rust import add_dep_helper
    from contextlib import ExitStack

    F32 = mybir.dt.float32
    F16 = mybir.dt.float16
    I32 = mybir.dt.int32
    I16 = mybir.dt.int16
    tok = nblocks * 128

    GB = 8   # groups per feature DMA batch
    GP_TS = False  # GPSIMD tensor_scalar measured ~3x slower: keep off
    nc = bacc.Bacc()
    table = nc.dram_tensor("table", [128, nchunk * 2 * C], F16, kind="ExternalInput")
    localseg = nc.dram_tensor("localseg", [128, nchunk], F32, kind="ExternalInput")
    iota_in = nc.dram_tensor("iota", [128, 128], F16, kind="ExternalInput")
    meta_off = nc.dram_tensor("meta_off", [1, ngroups], I32, kind="ExternalInput")
    scat_idx = nc.dram_tensor("scat_idx", [128, tok // 16], I16, kind="ExternalInput")
    out = nc.dram_tensor("out", [span_pad, C], F32, kind="ExternalOutput")

    with tile.TileContext(nc) as tc, ExitStack() as ctx:
        const = ctx.enter_context(tc.tile_pool(name="const", bufs=1))
        featp = ctx.enter_context(tc.tile_pool(name="feat", bufs=3))  # 3 x 8KB/part
        ohp = ctx.enter_context(tc.tile_pool(name="oh", bufs=2 * ABS_EVERY * V))
        psump = ctx.enter_context(tc.tile_pool(name="psum", bufs=1, space="PSUM"))
        absp = ctx.enter_context(tc.tile_pool(name="abs", bufs=4))

        iota_sb = const.tile([128, 128], F16)
        nc.sync.dma_start(iota_sb[:], iota_in[:])
        lseg_sb = const.tile([128, nchunk], F32)
        nc.sync.dma_start(lseg_sb[:], localseg[:])
        moff_sb = const.tile([1, ngroups], I32)
        nc.sync.dma_start(moff_sb[:], meta_off[:])
        idx_sb = const.tile([128, tok // 16], I16)
        nc.sync.dma_start(idx_sb[:], scat_idx[:])

        psum_acc = psump.tile([128, nblocks * 128], F32)
        pdum = psump.tile([128, 64], F32, tag="pdum")

        # K=1 start=True dummy matmuls zero the accumulator (and set
        # has_written) while keeping every PSUM dependency on the PE.
        zrow = const.tile([1, 640], F16)
        nc.vector.memset(zrow[:], 0.0)
        total = nblocks * 128
        pos = 0
        while pos < total:
            n = min(512, total - pos)
            nc.tensor.matmul(psum_acc[:, pos:pos + n], zrow[:, 512:512 + 128],
                             zrow[:, 0:n], start=True, stop=True,
                             skip_group_check=True)
            pos += n

        # Pre-touch consts on DVE / Pool so TensorScalarPtr & co. never need
        # more than one cross-engine wait (walrus 1-wait limit per compute op).
        scr16 = const.tile([128, 1], F16)
        nc.vector.tensor_copy(scr16[:], iota_sb[:, 0:1])
        scr32 = const.tile([128, 1], F32)
        nc.vector.tensor_copy(scr32[:], lseg_sb[:, 0:1])
        scrg = const.tile([128, 1], I16)
        nc.gpsimd.tensor_copy(scrg[:], idx_sb[:, 0:1])
        if GP_TS:
            scr16g = const.tile([128, 1], F16)
            nc.gpsimd.tensor_copy(scr16g[:], iota_sb[:, 0:1])
            scr32g = const.tile([128, 1], F32)
            nc.gpsimd.tensor_copy(scr32g[:], lseg_sb[:, 0:1])

        absorber = None
        feat = None
        offs = None
        CW = 2 * C                      # fp16 elems per chunk per partition
        for g in range(ngroups):
            if g % GB == 0:
                nb = min(GB, ngroups - g)
                feat = featp.tile([128, GB * V * CW], F16)
                nc.sync.dma_start(
                    feat[:, :nb * V * CW],
                    table[:, g * V * CW:(g + nb) * V * CW])
                _, offs = nc.values_load_multi_w_load_instructions(
                    moff_sb[0:1, g:g + nb],
                    engines=[mybir.EngineType.PE],
                    min_val=0, max_val=(nblocks - 1) * 128,
                    skip_runtime_bounds_check=True)
            if g % ABS_EVERY == 0 and g > 0:
                # PE -> DVE progress signal through an isolated PSUM bank:
                # later one-hot builds order after it so their tile-reuse WAR
                # waits are already-observed PE ticks (elided by Tile).
                nc.tensor.matmul(pdum[:, 0:64], zrow[:, 512:512 + 128],
                                 zrow[:, 0:64], start=True, stop=True,
                                 skip_group_check=True)
                abst = absp.tile([1, 1], F32)
                absorber = nc.vector.tensor_copy(abst[:], pdum[0:1, 0:1])

            dst = psum_acc[:, bass.ds(offs[g % GB], 128)]
            for v in range(V):
                c = g * V + v
                use_gp = GP_TS and (c % 2 == 1)
                oh = ohp.tile([128, 128], F16,
                              tag="ohg" if use_gp else "oh")
                eng = nc.gpsimd if use_gp else nc.vector
                ts = eng.tensor_scalar(
                    oh[:], iota_sb[:], lseg_sb[:, c:c + 1], None,
                    mybir.AluOpType.is_equal,
                )
                if absorber is not None:
                    add_dep_helper(ts.ins, absorber.ins, sync=False,
                                   reason="order TS after PE absorber")
                base = ((g % GB) * V + v) * CW
                nc.tensor.matmul(
                    dst, oh[:], feat[:, base:base + CW],
                    start=False, stop=True, skip_group_check=True,
                )

        stage = const.tile([128, nblocks * 64], F32)
        hi_v = psum_acc[:].rearrange("p (j two c) -> p j two c", two=2, c=C)
        nc.scalar.copy(stage[:].rearrange("p (j c) -> p j c", c=C), hi_v[:, :, 0, :])
        lo_v = stage[:].rearrange("p (j c) -> p j c", c=C)
        nc.vector.tensor_add(lo_v, lo_v, hi_v[:, :, 1, :])
        nc.gpsimd.dma_scatter_add(
            out[:],
            stage[:].rearrange("p (j c) -> p j c", c=C),
            idx_sb[:],
            tok,
            tok,
            C,
            single_packet=False,
        )
    nc.finalize()
    return nc


# ---------------- entry point ----------------
def kernel(image_feature, post_trans, post_rots, intrinsics, extrinsics,
           frustum, bev_res, bev_start_pos):
    from concourse.bass_utils import run_bass_kernel_spmd
    import os

    rank, mask = _compute_ranks(frustum, post_trans, post_rots, intrinsics,
                                extrinsics, bev_res, bev_start_pos)
    feats = np.ascontiguousarray(np.asarray(image_feature, np.float32)
                                 .reshape(B, NP_SAMPLE, C))
    plans, nchunk, ngroups, nblocks, span_pad = _plan_cores(rank, mask, feats)

    in_maps = [
        _build_inputs(pl, feats[pl.sample], nchunk, ngroups, nblocks, span_pad)
        for pl in plans
    ]

    key = (nchunk, ngroups, nblocks, span_pad)
    if key not in _compiled:
        _compiled[key] = _build_kernel(*key)
    nc = _compiled[key]

    trace = bool(int(os.environ.get("BEV_TRACE", "0")))
    res = run_bass_kernel_spmd(nc, in_maps, core_ids=list(range(NCORES)),
                               trace=trace,
                               trace_cores=[0] if trace else None)
    if trace and res.exec_time_ns is not None:
        print(f"HW exec time: {res.exec_time_ns} ns")
        kernel.last_exec_time_ns = res.exec_time_ns
        kernel.last_results = res

    grid = np.zeros((B, NBINS, C), np.float32)
    for k, pl in enumerate(plans):
        o = res.results[k]["out"]
        grid[pl.sample, pl.lo:pl.lo + pl.span] = o[:pl.span]
    return np.ascontiguousarray(
        grid.reshape(B, X, Y, C).transpose(0, 3, 1, 2))



# revision 2
# speedup vs baseline: 1.8778x; 1.8778x over previous
"""BEV voxel-pooling (segment_reduce) kernel for 8 Trainium2 NeuronCores.

Strategy (v2: row-aligned accumulation — no one-hot, no scatter)
----------------------------------------------------------------
Host (numpy, cheap — driven only by the small geometry inputs):
  * compute each point's BEV rank (bin id) exactly as the reference does
  * per sample: split each rank's point list into pseudo-segments of at
    most L=32 points; sort pseudo-segments by size (desc) and deal them
    round-robin onto the sample's 4 cores (8 cores total for B=2)
  * per core: group its pseudo-segments (still size-desc) into blocks of
    128; block b needs K_b = size of its largest member chunks. Assign
    pseudo-segment j to PSUM partition row j%128 of block j//128.
  * pack features into chunks: chunk (b, k) partition p holds the k-th
    point of pseudo-segment (b, p) as fp16, or zeros past its size. The
    per-block chunk counts K_b are envelope-maxed across all 8 cores so
    a single SPMD program serves every core (zero rows add nothing).

Device (per core, one SPMD Bass/Tile program):
  * load a 128x128 fp16 identity into the PE once (explicit ldweights;
    every matmul is marked non-self-loading), then for each chunk run
    matmul(psum_block, I, chunk) with start on k==0 / stop on k==K_b-1:
    PSUM partition p of block b accumulates the f32 sum of pseudo-seg
    (b, p). No DVE work at all; the whole run is feature-DMA bound.
  * when block b completes, ACT-copy its [128, 64] PSUM window to SBUF
    (fp16) and DMA it to out[b*128:(b+1)*128] — dense rows, no scatter.

Host gather: out row j of a core is pseudo-segment j's sum; np.add.at
into the (B, 40000, 64) grid by each pseudo-segment's rank, reshape to
the reference layout (B, C, X, Y).
"""
import sys
sys.path.insert(0, '/opt/trn_rl_repo')

import numpy as np

# ---------------- problem constants (hardcoded per spec) ----------------
B, N, C = 2, 6, 64
H_IMG, W_IMG = 256, 704
DS = 16
DSH, DSW = H_IMG // DS, W_IMG // DS          # 16, 44
D0, D1 = 4, 45                                # depth bins -> D = 41
X, Y, Z = 200, 200, 1
NBINS = X * Y * Z                             # 40000
NP_SAMPLE = N * (D1 - D0) * DSH * DSW         # 173184
NCORES = 8
SHARDS_PER_SAMPLE = 4

L = 32        # max points per pseudo-segment
BC = 16       # chunks per feature DMA batch

_compiled = {}


# ---------------- host geometry (matches reference numerics) ----------------
def _compute_ranks(frustum, post_trans, post_rots, intrinsics, extrinsics,
                   bev_res, bev_start_pos):
    frustum = np.asarray(frustum, np.float32)
    post_trans = np.asarray(post_trans, np.float32)
    post_rots = np.asarray(post_rots, np.float32)
    intrinsics = np.asarray(intrinsics, np.float32)
    extrinsics = np.asarray(extrinsics, np.float32)
    bev_res = np.asarray(bev_res, np.float32)
    bev_start_pos = np.asarray(bev_start_pos, np.float32)

    ext_inv = np.linalg.inv(extrinsics.astype(np.float64)).astype(np.float32)
    rot = ext_inv[..., :3, :3]
    trans = ext_inv[..., :3, 3]
    pts = frustum[None, None] - post_trans[:, :, None, None, None, :]
    pr_inv = np.linalg.inv(post_rots.astype(np.float64)).astype(np.float32)
    pts = np.einsum('bnij,bndhwj->bndhwi', pr_inv, pts).astype(np.float32)
    pts = np.concatenate([pts[..., :2] * pts[..., 2:3], pts[..., 2:3]], axis=-1)
    comb = (rot @ np.linalg.inv(intrinsics.astype(np.float64)).astype(np.float32)
            ).astype(np.float32)
    pts = np.einsum('bnij,bndhwj->bndhwi', comb, pts).astype(np.float32)
    geom = pts + trans[:, :, None, None, None, :]

    coords = (geom - (bev_start_pos - bev_res / 2.0)) / bev_res
    ci = coords.reshape(B, -1, 3).astype(np.int32)
    mask = ((ci[..., 0] >= 0) & (ci[..., 0] < X) &
            (ci[..., 1] >= 0) & (ci[..., 1] < Y) &
            (ci[..., 2] >= 0) & (ci[..., 2] < Z))
    rank = ci[..., 0] * (Y * Z) + ci[..., 1] * Z + ci[..., 2]
    return rank, mask


# ---------------- host planning ----------------
def _plan_cores(rank, mask):
    """Split every sample's ranks into <=L-point pseudo-segments, deal them
    round-robin (by desc size) onto 4 cores each; compute the cross-core
    block profile (NB, K_b)."""
    cores = []
    for b in range(B):
        r = rank[b]
        m = mask[b]
        valid = np.nonzero(m)[0]
        order = valid[np.argsort(r[valid], kind='stable')]
        rs = r[order]
        newseg = np.r_[True, rs[1:] != rs[:-1]]
        seg_start = np.nonzero(newseg)[0]
        seg_rank = rs[seg_start]
        seg_cnt = np.diff(np.r_[seg_start, len(rs)])
        nseg = len(seg_start)

        npieces = (seg_cnt + L - 1) // L
        piece_seg = np.repeat(np.arange(nseg), npieces)
        piece_off = np.arange(len(piece_seg)) - np.repeat(
            np.cumsum(npieces) - npieces, npieces)
        piece_start = seg_start[piece_seg] + piece_off * L
        piece_cnt = np.minimum(seg_cnt[piece_seg] - piece_off * L, L).astype(np.int64)
        piece_rank = seg_rank[piece_seg]

        po = np.argsort(-piece_cnt, kind='stable')
        for c in range(SHARDS_PER_SAMPLE):
            sel = po[c::SHARDS_PER_SAMPLE]
            cores.append(dict(
                sample=b,
                start=piece_start[sel],
                cnt=piece_cnt[sel],
                rank=piece_rank[sel],
                order=order,
            ))

    NB = max((len(c['cnt']) + 127) // 128 for c in cores)
    Kb = np.ones(NB, np.int64)
    for c in cores:
        cnt = c['cnt']
        for j in range((len(cnt) + 127) // 128):
            Kb[j] = max(Kb[j], int(cnt[j * 128]))
    base = np.concatenate([[0], np.cumsum(Kb)])[:-1]
    NC = int(Kb.sum())
    return cores, NB, Kb, base, NC


def _build_table(core, feats16_b, NB, Kb, base, NC):
    """Per-core packed feature table [128, NC*C] fp16 (partition-major)."""
    tbl = np.zeros((NC, 128, C), np.float16)
    cnt = core['cnt']
    start = core['start']
    order = core['order']
    n = len(cnt)
    if n:
        seg_ids = np.arange(n)
        blk = seg_ids // 128
        row = seg_ids % 128
        tot = int(cnt.sum())
        pt_seg = np.repeat(seg_ids, cnt)
        within = np.arange(tot) - np.repeat(np.cumsum(cnt) - cnt, cnt)
        src = order[np.repeat(start, cnt) + within]
        chunk = base[blk[pt_seg]] + within
        tbl[chunk, row[pt_seg]] = feats16_b[src]
    return np.ascontiguousarray(tbl.transpose(1, 0, 2).reshape(128, NC * C))


# ---------------- device program ----------------
def _build_kernel(NB, Kb, NC):
    import concourse.bass as bass
    import concourse.bacc as bacc
    import concourse.mybir as mybir
    import concourse.tile as tile
    from contextlib import ExitStack

    F32 = mybir.dt.float32
    F16 = mybir.dt.float16

    nc = bacc.Bacc()
    table = nc.dram_tensor("table", [128, NC * C], F16, kind="ExternalInput")
    ident = nc.dram_tensor("ident", [128, 128], F16, kind="ExternalInput")
    out = nc.dram_tensor("out", [NB * 128, C], F16, kind="ExternalOutput")

    with tile.TileContext(nc) as tc, ExitStack() as ctx:
        const = ctx.enter_context(tc.tile_pool(name="const", bufs=1))
        featp = ctx.enter_context(tc.tile_pool(name="feat", bufs=3))
        stagep = ctx.enter_context(tc.tile_pool(name="stage", bufs=4))
        psump = ctx.enter_context(tc.tile_pool(name="psum", bufs=1, space="PSUM"))

        ident_sb = const.tile([128, 128], F16)
        nc.sync.dma_start(ident_sb[:], ident[:])
        acc = psump.tile([128, NB * C], F32)

        nc.tensor.ldweights(ident_sb[:])

        t = 0
        feat = None
        for b in range(NB):
            kb = int(Kb[b])
            for k in range(kb):
                if t % BC == 0:
                    nb_ = min(BC, NC - t)
                    feat = featp.tile([128, BC * C], F16)
                    nc.sync.dma_start(feat[:, :nb_ * C],
                                      table[:, t * C:(t + nb_) * C])
                mm = nc.tensor.matmul(
                    acc[:, b * C:(b + 1) * C], ident_sb[:],
                    feat[:, (t % BC) * C:(t % BC + 1) * C],
                    start=(k == 0), stop=(k == kb - 1),
                    skip_group_check=True)
                mm.ins.ldweights = False
                t += 1
            st = stagep.tile([128, C], F16)
            nc.scalar.copy(st[:], acc[:, b * C:(b + 1) * C])
            nc.sync.dma_start(out[b * 128:(b + 1) * 128, :], st[:])
    nc.finalize()
    return nc


# ---------------- entry point ----------------
def kernel(image_feature, post_trans, post_rots, intrinsics, extrinsics,
           frustum, bev_res, bev_start_pos):
    from concourse.bass_utils import run_bass_kernel_spmd
    import os

    rank, mask = _compute_ranks(frustum, post_trans, post_rots, intrinsics,
                                extrinsics, bev_res, bev_start_pos)
    feats16 = np.asarray(image_feature, np.float32).reshape(
        B, NP_SAMPLE, C).astype(np.float16)
    cores, NB, Kb, base, NC = _plan_cores(rank, mask)

    ident = np.eye(128, dtype=np.float16)
    in_maps = [
        {"table": _build_table(c, feats16[c['sample']], NB, Kb, base, NC),
         "ident": ident}
        for c in cores
    ]

    key = (NB, tuple(int(k) for k in Kb), NC)
    if key not in _compiled:
        _compiled[key] = _build_kernel(NB, Kb, NC)
    nc = _compiled[key]

    trace = bool(int(os.environ.get("BEV_TRACE", "0")))
    res = run_bass_kernel_spmd(nc, in_maps, core_ids=list(range(NCORES)),
                               trace=trace,
                               trace_cores=[0] if trace else None)
    if trace and res.exec_time_ns is not None:
        print(f"HW exec time: {res.exec_time_ns} ns")
        kernel.last_exec_time_ns = res.exec_time_ns
        kernel.last_results = res

    grid = np.zeros((B, NBINS, C), np.float32)
    for ci, core in enumerate(cores):
        o = np.asarray(res.results[ci]["out"], np.float32)
        n = len(core['cnt'])
        if n:
            np.add.at(grid[core['sample']], core['rank'], o[:n])
    return np.ascontiguousarray(
        grid.reshape(B, X, Y, C).transpose(0, 3, 1, 2))


# revision 5
# speedup vs baseline: 2.4344x; 1.2964x over previous
"""BEV voxel-pooling (segment_reduce) kernel for 8 Trainium2 NeuronCores.

Strategy (v2: row-aligned accumulation — no one-hot, no scatter)
----------------------------------------------------------------
Host (numpy, cheap — driven only by the small geometry inputs):
  * compute each point's BEV rank (bin id) exactly as the reference does
  * per sample: split each rank's point list into pseudo-segments of at
    most L=32 points; sort pseudo-segments by size (desc) and deal them
    round-robin onto the sample's 4 cores (8 cores total for B=2)
  * per core: group its pseudo-segments (still size-desc) into blocks of
    128; block b needs K_b = size of its largest member chunks. Assign
    pseudo-segment j to PSUM partition row j%128 of block j//128.
  * pack features into chunks: chunk (b, k) partition p holds the k-th
    point of pseudo-segment (b, p) as fp16, or zeros past its size. The
    per-block chunk counts K_b are envelope-maxed across all 8 cores so
    a single SPMD program serves every core (zero rows add nothing).

Device (per core, one SPMD Bass/Tile program):
  * load a 128x128 fp16 identity into the PE once (explicit ldweights;
    every matmul is marked non-self-loading), then for each chunk run
    matmul(psum_block, I, chunk) with start on k==0 / stop on k==K_b-1:
    PSUM partition p of block b accumulates the f32 sum of pseudo-seg
    (b, p). No DVE work at all; the whole run is feature-DMA bound.
  * when block b completes, ACT-copy its [128, 64] PSUM window to SBUF
    (fp16) and DMA it to out[b*128:(b+1)*128] — dense rows, no scatter.

Host gather: out row j of a core is pseudo-segment j's sum; np.add.at
into the (B, 40000, 64) grid by each pseudo-segment's rank, reshape to
the reference layout (B, C, X, Y).
"""
import sys
sys.path.insert(0, '/opt/trn_rl_repo')

import numpy as np

# ---------------- problem constants (hardcoded per spec) ----------------
B, N, C = 2, 6, 64
H_IMG, W_IMG = 256, 704
DS = 16
DSH, DSW = H_IMG // DS, W_IMG // DS          # 16, 44
D0, D1 = 4, 45                                # depth bins -> D = 41
X, Y, Z = 200, 200, 1
NBINS = X * Y * Z                             # 40000
NP_SAMPLE = N * (D1 - D0) * DSH * DSW         # 173184
NCORES = 8
SHARDS_PER_SAMPLE = 4

L = 32        # max points per pseudo-segment
BC = 32       # chunks per feature DMA batch

_compiled = {}


# ---------------- host geometry (matches reference numerics) ----------------
def _compute_ranks(frustum, post_trans, post_rots, intrinsics, extrinsics,
                   bev_res, bev_start_pos):
    frustum = np.asarray(frustum, np.float32)
    post_trans = np.asarray(post_trans, np.float32)
    post_rots = np.asarray(post_rots, np.float32)
    intrinsics = np.asarray(intrinsics, np.float32)
    extrinsics = np.asarray(extrinsics, np.float32)
    bev_res = np.asarray(bev_res, np.float32)
    bev_start_pos = np.asarray(bev_start_pos, np.float32)

    ext_inv = np.linalg.inv(extrinsics.astype(np.float64)).astype(np.float32)
    rot = ext_inv[..., :3, :3]
    trans = ext_inv[..., :3, 3]
    pts = frustum[None, None] - post_trans[:, :, None, None, None, :]
    pr_inv = np.linalg.inv(post_rots.astype(np.float64)).astype(np.float32)
    pts = np.einsum('bnij,bndhwj->bndhwi', pr_inv, pts).astype(np.float32)
    pts = np.concatenate([pts[..., :2] * pts[..., 2:3], pts[..., 2:3]], axis=-1)
    comb = (rot @ np.linalg.inv(intrinsics.astype(np.float64)).astype(np.float32)
            ).astype(np.float32)
    pts = np.einsum('bnij,bndhwj->bndhwi', comb, pts).astype(np.float32)
    geom = pts + trans[:, :, None, None, None, :]

    coords = (geom - (bev_start_pos - bev_res / 2.0)) / bev_res
    ci = coords.reshape(B, -1, 3).astype(np.int32)
    mask = ((ci[..., 0] >= 0) & (ci[..., 0] < X) &
            (ci[..., 1] >= 0) & (ci[..., 1] < Y) &
            (ci[..., 2] >= 0) & (ci[..., 2] < Z))
    rank = ci[..., 0] * (Y * Z) + ci[..., 1] * Z + ci[..., 2]
    return rank, mask


# ---------------- host planning ----------------
def _plan_cores(rank, mask):
    """Split every sample's ranks into <=L-point pseudo-segments, deal them
    round-robin (by desc size) onto 4 cores each; compute the cross-core
    block profile (NB, K_b)."""
    cores = []
    for b in range(B):
        r = rank[b]
        m = mask[b]
        valid = np.nonzero(m)[0]
        order = valid[np.argsort(r[valid], kind='stable')]
        rs = r[order]
        newseg = np.r_[True, rs[1:] != rs[:-1]]
        seg_start = np.nonzero(newseg)[0]
        seg_rank = rs[seg_start]
        seg_cnt = np.diff(np.r_[seg_start, len(rs)])
        nseg = len(seg_start)

        npieces = (seg_cnt + L - 1) // L
        piece_seg = np.repeat(np.arange(nseg), npieces)
        piece_off = np.arange(len(piece_seg)) - np.repeat(
            np.cumsum(npieces) - npieces, npieces)
        piece_start = seg_start[piece_seg] + piece_off * L
        piece_cnt = np.minimum(seg_cnt[piece_seg] - piece_off * L, L).astype(np.int64)
        piece_rank = seg_rank[piece_seg]

        po = np.argsort(-piece_cnt, kind='stable')
        for c in range(SHARDS_PER_SAMPLE):
            sel = po[c::SHARDS_PER_SAMPLE]
            cores.append(dict(
                sample=b,
                start=piece_start[sel],
                cnt=piece_cnt[sel],
                rank=piece_rank[sel],
                order=order,
            ))

    NB = max((len(c['cnt']) + 127) // 128 for c in cores)
    Kb = np.ones(NB, np.int64)
    for c in cores:
        cnt = c['cnt']
        for j in range((len(cnt) + 127) // 128):
            Kb[j] = max(Kb[j], int(cnt[j * 128]))
    base = np.concatenate([[0], np.cumsum(Kb)])[:-1]
    NC = int(Kb.sum())
    return cores, NB, Kb, base, NC


def _build_table(core, feats16_b, NB, Kb, base, NC):
    """Per-core packed feature table [128, NC*C] fp16 (partition-major)."""
    tbl = np.zeros((NC, 128, C), np.float16)
    cnt = core['cnt']
    start = core['start']
    order = core['order']
    n = len(cnt)
    if n:
        seg_ids = np.arange(n)
        blk = seg_ids // 128
        row = seg_ids % 128
        tot = int(cnt.sum())
        pt_seg = np.repeat(seg_ids, cnt)
        within = np.arange(tot) - np.repeat(np.cumsum(cnt) - cnt, cnt)
        src = order[np.repeat(start, cnt) + within]
        chunk = base[blk[pt_seg]] + within
        tbl[chunk, row[pt_seg]] = feats16_b[src]
    return np.ascontiguousarray(tbl.transpose(1, 0, 2).reshape(128, NC * C))


# ---------------- device program ----------------
def _build_kernel(NB, Kb, NC):
    import concourse.bass as bass
    import concourse.bacc as bacc
    import concourse.mybir as mybir
    import concourse.tile as tile
    from contextlib import ExitStack

    F32 = mybir.dt.float32
    F16 = mybir.dt.float16

    nc = bacc.Bacc()
    table = nc.dram_tensor("table", [128, NC * C], F16, kind="ExternalInput")
    ident = nc.dram_tensor("ident", [128, 128], F16, kind="ExternalInput")
    out = nc.dram_tensor("out", [NB * 128, C], F16, kind="ExternalOutput")

    with tile.TileContext(nc) as tc, ExitStack() as ctx:
        const = ctx.enter_context(tc.tile_pool(name="const", bufs=1))
        featp = ctx.enter_context(tc.tile_pool(name="feat", bufs=3))
        stagep = ctx.enter_context(tc.tile_pool(name="stage", bufs=4))
        psump = ctx.enter_context(tc.tile_pool(name="psum", bufs=6, space="PSUM"))

        ident_sb = const.tile([128, 128], F16)
        nc.sync.dma_start(ident_sb[:], ident[:])

        nc.tensor.ldweights(ident_sb[:])

        t = 0
        feat = None
        for b in range(NB):
            kb = int(Kb[b])
            accb = psump.tile([128, C], F32, tag="acc")
            for k in range(kb):
                if t % BC == 0:
                    nb_ = min(BC, NC - t)
                    feat = featp.tile([128, BC * C], F16)
                    nc.sync.dma_start(feat[:, :nb_ * C],
                                      table[:, t * C:(t + nb_) * C])
                nc.tensor.matmul(
                    accb[:], ident_sb[:],
                    feat[:, (t % BC) * C:(t % BC + 1) * C],
                    start=(k == 0), stop=(k == kb - 1),
                    skip_group_check=True)
                t += 1
            st = stagep.tile([128, C], F16)
            nc.vector.tensor_copy(st[:], accb[:])
            nc.scalar.dma_start(out[b * 128:(b + 1) * 128, :], st[:])
    nc.finalize()
    return nc


# ---------------- entry point ----------------
def kernel(image_feature, post_trans, post_rots, intrinsics, extrinsics,
           frustum, bev_res, bev_start_pos):
    from concourse.bass_utils import run_bass_kernel_spmd
    import os

    rank, mask = _compute_ranks(frustum, post_trans, post_rots, intrinsics,
                                extrinsics, bev_res, bev_start_pos)
    feats16 = np.asarray(image_feature, np.float32).reshape(
        B, NP_SAMPLE, C).astype(np.float16)
    cores, NB, Kb, base, NC = _plan_cores(rank, mask)

    ident = np.eye(128, dtype=np.float16)
    in_maps = [
        {"table": _build_table(c, feats16[c['sample']], NB, Kb, base, NC),
         "ident": ident}
        for c in cores
    ]

    key = (NB, tuple(int(k) for k in Kb), NC)
    if key not in _compiled:
        _compiled[key] = _build_kernel(NB, Kb, NC)
    nc = _compiled[key]

    trace = bool(int(os.environ.get("BEV_TRACE", "0")))
    res = run_bass_kernel_spmd(nc, in_maps, core_ids=list(range(NCORES)),
                               trace=trace,
                               trace_cores=[0] if trace else None)
    if trace and res.exec_time_ns is not None:
        print(f"HW exec time: {res.exec_time_ns} ns")
        kernel.last_exec_time_ns = res.exec_time_ns
        kernel.last_results = res

    grid = np.zeros((B, NBINS, C), np.float32)
    for ci, core in enumerate(cores):
        o = np.asarray(res.results[ci]["out"], np.float32)
        n = len(core['cnt'])
        if n:
            np.add.at(grid[core['sample']], core['rank'], o[:n])
    return np.ascontiguousarray(
        grid.reshape(B, X, Y, C).transpose(0, 3, 1, 2))
